# revision 18
# baseline (speedup 1.0000x reference)
"""Hashed-weight MLP (1024-4096-4096-32000, batch 2048) on 8 TRN2 NeuronCores.

Problem: h = relu(x @ W0); h = relu(h @ W1); out = h @ W2, where each
W_l[i, j] = hw_l[(a_l*i + b_l*j + c_l) % N_l] is a virtual (ROBE-Z hashed)
weight gathered from a small parameter vector.

Approach (column-parallel tensor parallelism on all three layers):
  * Via the host-permuted table hb_bb[t] = hw[(b*t) % N] the virtual weight
    becomes row-contiguous: W[i, col] = hb_bb[shift + q*kk + r*c1 + col] with
    i = k*c1 + kk (q = b^-1 a, r = signed residue of q*k mod N). W2 tiles are
    DMAd straight from the per-core table slice into SBUF, 2-3 strided DMAs
    per 128-row tile (head run / kk-outer full-block rect / tail run).
  * Each core owns a 1/8 column shard of every layer; shard offsets are
    absorbed into the host-side slice (SPMD-uniform device program).
  * L2 uses uniform 128-row k-tiles whose within-tile row order is the
    ladder's kk-outer permutation. The matching rhs permutation is absorbed
    into WHICH h2 columns each core computes in L1 (host-materialized W1
    simply picks the permuted column subset), so the AllGathered h2 lands in
    DRAM already tile-major and h2res loads are plain 3-level strided DMAs.
  * L1 is contraction-parallel: each core multiplies its OWN L0 output
    (kept in SBUF, h1 never touches DRAM) against its 512-row slice of W1
    over all 4096 (permuted) output columns, then one AllReduce(+add) per
    batch tile sums the partials into the full pre-relu h2 on every core
    (relu applied later in-place on the SBUF-resident h2). This removes
    the h1 AllGathers entirely and halves the collective op count.
  * L2 keeps the whole 4096 x 2048 h2 activation resident in SBUF and
    streams W2 tiles through a ring.
  * Engines: scalar/sync = weight ladders + partial stores + output
    stores; vector/gpsimd = ReLU + PSUM evacuation (bf16 cast); gpsimd
    also triggers collectives and loads x/h2res. GEMMs are bf16 with
    fp32 PSUM; partials are reduced in bf16.
"""
import sys
if "/opt/trn_rl_repo" not in sys.path:
    sys.path.insert(0, "/opt/trn_rl_repo")

import numpy as np
import ml_dtypes

import concourse.bass as bass
import concourse.bacc as bacc
import concourse.tile as tile
import concourse.mybir as mybir
from concourse.bass_utils import run_bass_kernel_spmd

N_CORES = 8
P = 128
NB = 512                      # batch tile (matmul moving dim)
BATCH = 2048
BT = BATCH // NB              # 4

LENS = [1024, 4096, 4096, 32000]
HASH_A = [9973, 10007, 10039]
HASH_B = [31013, 31019, 31039]
HASH_C = [557, 563, 569]
SIZES = [1048576, 1048576, 4194304]
JW = [512, 512, 4000]         # true per-core output shard width
WTOT = [512, 512, 4096]       # max ladder col offset (L2 incl. jg offsets)

BF = mybir.dt.bfloat16
F32 = mybir.dt.float32


def _plan_layer(l):
    N = SIZES[l]; a, b, ch = HASH_A[l], HASH_B[l], HASH_C[l]
    binv = pow(b, -1, N)
    q = (binv * a) % N
    u0 = (binv * ch) % N
    in_dim = LENS[l]
    best = None
    for k in range(1, min(in_dim, 600) + 1):
        r = (q * k) % N
        if r > N // 2:
            r -= N
        C1 = -(-in_dim // k)
        extra = q * (k - 1) + abs(r) * (C1 - 1)
        if best is None or extra < best[0]:
            best = (extra, k, C1, r)
    _, k, C1, r = best
    shift = max(0, -r * (C1 - 1))
    m_ext = shift + q * (k - 1) + max(r, 0) * (C1 - 1) + WTOT[l] + 64
    return dict(N=N, b=b, q=q, u0=u0, k=k, r=r, shift=shift,
                m_ext=m_ext, in_dim=in_dim)


PLANS = [_plan_layer(l) for l in range(3)]
RG = [list(range(N_CORES))]


def _l2_tiles():
    """Uniform 128-row tiles for L2; seg = (kk0, c1_0, kc, c1c, p0).
    Within a multi-block seg the DMA streams kk-outer, so partition
    p = p0 + kkrel*c1c + c1rel."""
    k = PLANS[2]["k"]
    tiles = []
    for t in range(LENS[2] // P):
        i0, h = P * t, P
        segs = []
        row = i0
        kk0 = row % k
        if kk0:
            cnt = min(k - kk0, h)
            segs.append((kk0, row // k, cnt, 1, 0))
            row += cnt
        nfull = (i0 + h - row) // k
        if nfull:
            segs.append((0, row // k, k, nfull, row - i0))
            row += nfull * k
        if row < i0 + h:
            segs.append((0, row // k, i0 + h - row, 1, row - i0))
        tiles.append((i0, h, segs))
    return tiles


LT2 = _l2_tiles()
NKT2 = len(LT2)               # 32
NJG2 = 8                      # L2 j-groups of width 512 (4 j-tiles of 128)

def _l2_perm():
    """Global h2-row permutation: PERM[pos] = natural contraction row stored
    at tile-major position pos (tile t occupies positions [128t, 128t+128))."""
    perm = []
    for (i0, h, segs) in LT2:
        order = {}
        for (kk0, c1_0, kc, c1c, p0) in segs:
            for kk in range(kc):
                for c1r in range(c1c):
                    order[p0 + kk * c1c + c1r] = (c1_0 + c1r) * PLANS[2]["k"] \
                        + kk0 + kk
        perm.extend(order[p] for p in range(h))
    return np.array(perm, dtype=np.int64)


PERM2 = _l2_perm()


def _ladder_dmas(nc, eng, hb_t, l, wtile_ap, segs, col0, w):
    """Emit ladder DMAs for one weight tile of layer l into SBUF tile."""
    pl = PLANS[l]
    q, r, shift = pl["q"], pl["r"], pl["shift"]
    for (kk0, c1_0, kc, c1c, p0) in segs:
        if c1c == 1:
            src = bass.AP(hb_t, shift + q * kk0 + r * c1_0 + col0,
                          [[q, kc], [1, w]])
        else:
            src = bass.AP(hb_t, shift + q * kk0 + r * c1_0 + col0,
                          [[q, kc], [r, c1c], [1, w]])
        eng.dma_start(out=wtile_ap[p0:p0 + kc * c1c, :], in_=src)


def build_nc():
    nc = bacc.Bacc("TRN2", target_bir_lowering=False, debug=False,
                   num_devices=N_CORES)

    # x host-tiled as [128, BT*8*512]: row p holds, for each batch tile b
    # and k-tile kt, the 512 batch entries of x[kt*128+p, b*512:...], so a
    # per-(b) load is one plain 2D strided DMA with 8KB rows.
    xm_d = nc.dram_tensor("xm", [P, BT * 8 * NB], BF,
                          kind="ExternalInput").ap()
    # W0/W1 are host-materialized tile-major (wide rows -> 8KB descriptors);
    # only the big W2 still streams via the on-device hash ladder.
    w0m_d = nc.dram_tensor("w0m", [128, 4096], BF, kind="ExternalInput").ap()
    w1m_d = nc.dram_tensor("w1m", [4, 128, 4096], BF,
                           kind="ExternalInput").ap()
    hb2 = nc.dram_tensor("hb2", [PLANS[2]["m_ext"]], BF,
                         kind="ExternalInput").ap()
    # per-b partial h2 (pre-relu, PERM2 column order) and the AllReduced
    # full pre-relu h2
    pt = [nc.dram_tensor(f"pt{b}", [4096, NB], BF).ap() for b in range(BT)]
    h2f = [nc.dram_tensor(f"h2f{b}", [4096, NB], BF, addr_space="Shared").ap()
           for b in range(BT)]
    out_d = nc.dram_tensor("outT", [4096, BATCH], BF, kind="ExternalOutput").ap()

    with tile.TileContext(nc) as tc, \
         tc.tile_pool(name="ps", bufs=8, space="PSUM") as psp, \
         tc.tile_pool(name="resid", bufs=1) as residp:

        # h2 stays fully SBUF-resident for L2 (128 KB/partition); one wide
        # tile per batch tile, loaded in 4 chunks from h2f (whose rows are
        # already in ladder-permuted tile-major order).
        h2res = [residp.tile([P, NKT2 * NB], BF, name=f"h2r_{b}")
                 for b in range(BT)]

        def load_h2res(b):
            # on gpsimd (SWDGE): third DGE lane, keeps both HWDGE rings
            # free; 4 chunks so collective DMAs can interleave
            # h2f carries pre-relu sums; relu in place after each chunk.
            # b0/b1 relu on vector (free until L2's first psum copy);
            # b2/b3 on gpsimd so their late-AG waits never block the
            # vector queue ahead of L2's psum evacuations.
            reng = nc.vector if b < 2 else nc.gpsimd
            with nc.allow_non_contiguous_dma(reason="perm act load"):
                for c in range(4):
                    sl = slice(c * 8 * NB, (c + 1) * 8 * NB)
                    nc.gpsimd.dma_start(
                        out=h2res[b][:, sl],
                        in_=bass.AP(h2f[b].tensor, c * 8 * P * NB,
                                    [[NB, P], [P * NB, 8], [1, NB]]))
                    reng.tensor_scalar_max(
                        h2res[b][:, sl], h2res[b][:, sl], 0.0)

        # head of the first L2 slab: own (outer) pool so its ladders run
        # during L0/L1 without write-after-read waits on dying pools
        NHEAD = 8
        with nc.allow_non_contiguous_dma(reason="hash ladder"), \
             tc.tile_pool(name="jg0h", bufs=1) as jg0hp:
            jg0head = [jg0hp.tile([P, 512], BF, name=f"w2h{kt}")
                       for kt in range(NHEAD)]

            # w1sb lives below the L0 pools; its ladders prefetch during L0
            with tc.tile_pool(name="l1w", bufs=1) as l1wp:
                w1sb = [l1wp.tile([128, 4096], BF, name=f"w1sb{t}")
                        for t in range(4)]

                # ---- Layer 0 + contraction-parallel Layer 1 ----
                with tc.tile_pool(name="l0", bufs=1) as l0p, \
                     tc.tile_pool(name="l0x", bufs=2) as l0xp, \
                     tc.tile_pool(name="l0h", bufs=8) as l0hp, \
                     tc.tile_pool(name="l1o", bufs=6) as l1op:
                    w0sb = l0p.tile([128, 4096], BF, name="w0sb")

                    def load_x(b, eng):
                        xt = l0xp.tile([P, 8 * NB], BF, tag="x",
                                       name=f"x_{b}")
                        eng.dma_start(
                            out=xt[:],
                            in_=xm_d[:, b * 8 * NB:(b + 1) * 8 * NB])
                        return xt

                    # critical startup path first: w0 on scalar, x0 on sync
                    nc.scalar.dma_start(out=w0sb[:], in_=w0m_d[:, :])
                    xq = [load_x(0, nc.sync), load_x(1, nc.scalar)]
                    for t in range(4):
                        eng = nc.sync if t % 2 == 0 else nc.scalar
                        eng.dma_start(out=w1sb[t][:], in_=w1m_d[t, :, :])
                    for kt in range(NHEAD):
                        eng = nc.scalar if kt % 2 == 0 else nc.sync
                        _ladder_dmas(nc, eng, hb2.tensor, 2,
                                     jg0head[kt][:], LT2[kt][2], 0, 512)

                    for b in range(BT):
                        # L0(b): 4 j-tiles = L1's local contraction k-tiles;
                        # they stay in SBUF as L1's rhs (h1 never hits DRAM)
                        hq = []
                        for j in range(4):
                            ps = psp.tile([P, NB], F32, tag="ps",
                                          name=f"ps0_{b}_{j}")
                            for kt in range(8):
                                nc.tensor.matmul(
                                    out=ps[:],
                                    lhsT=w0sb[:, kt * 512 + j * P:
                                              kt * 512 + (j + 1) * P],
                                    rhs=xq[b][:, kt * NB:(kt + 1) * NB],
                                    start=(kt == 0), stop=(kt == 7))
                            hsb = l0hp.tile([P, NB], BF, tag="h1",
                                            name=f"h1_{b}_{j}")
                            nc.vector.tensor_scalar_max(hsb[:], ps[:], 0.0)
                            hq.append(hsb)
                        if b + 2 < BT:
                            xq.append(load_x(b + 2, nc.gpsimd))
                        # L1 partial over the local 512-row contraction
                        # slice, all 4096 PERM2-ordered output columns
                        for jt in range(32):
                            ps = psp.tile([P, NB], F32, tag="ps",
                                          name=f"ps1_{b}_{jt}")
                            for kt in range(4):
                                nc.tensor.matmul(
                                    out=ps[:],
                                    lhsT=w1sb[kt][:, jt * P:(jt + 1) * P],
                                    rhs=hq[kt][:],
                                    start=(kt == 0), stop=(kt == 3))
                            ob = l1op.tile([P, NB], BF, tag="l1o",
                                           name=f"p_{b}_{jt}")
                            nc.vector.tensor_copy(out=ob[:], in_=ps[:])
                            seng = nc.sync if jt % 2 == 0 else nc.scalar
                            seng.dma_start(
                                out=pt[b][jt * P:(jt + 1) * P, :], in_=ob[:])
                        # one AllReduce per batch tile: partials sum
                        # straight into the full pre-relu h2 on every core
                        nc.gpsimd.collective_compute(
                            "AllReduce", mybir.AluOpType.add,
                            replica_groups=RG,
                            ins=[pt[b].opt()], outs=[h2f[b].opt()])
                        if b >= 2:
                            load_h2res(b - 2)

                    load_h2res(2)
                    load_h2res(3)

            # ---------------- Layer 2 ----------------
            # snake schedule: (jg0..jg7) x (b0,b1), then (jg7..jg0) x
            # (b2,b3). Only b0/b1's AllGather gates sit on the critical
            # path; b2/b3 arrive hundreds of us before first use. Each W2
            # slab is ladder-loaded twice (DMA has ample headroom), with
            # jg7's slab reused across the turn and jg0's head tiles
            # reused on both visits.
            runs = [(jg, (0, 1)) for jg in range(NJG2)] + \
                   [(jg, (2, 3)) for jg in reversed(range(NJG2))]
            with tc.tile_pool(name="w2", bufs=48) as w2p, \
                 tc.tile_pool(name="l2o", bufs=4) as l2op:
                slab = None
                last_jg = None
                for vi, (jg, bs) in enumerate(runs):
                    if jg != last_jg:
                        slab = []
                        for kt in range(NKT2):
                            if jg == 0 and kt < NHEAD:
                                slab.append(jg0head[kt])
                                continue
                            wt = w2p.tile([P, 512], BF, tag="w2t",
                                          name=f"w2_{vi}_{kt}")
                            weng = nc.scalar if kt % 2 == 0 else nc.sync
                            _ladder_dmas(nc, weng, hb2.tensor, 2, wt[:],
                                         LT2[kt][2], jg * 512, 512)
                            slab.append(wt)
                        last_jg = jg
                    for b in bs:
                        pss = [psp.tile([P, NB], F32, tag="ps",
                                        name=f"ps2_{vi}_{b}_{j}")
                               for j in range(4)]
                        for kt in range(NKT2):
                            for j in range(4):
                                nc.tensor.matmul(
                                    out=pss[j][:],
                                    lhsT=slab[kt][:, j * P:(j + 1) * P],
                                    rhs=h2res[b][:, kt * NB:(kt + 1) * NB],
                                    start=(kt == 0), stop=(kt == NKT2 - 1))
                        for j in range(4):
                            osb = l2op.tile([P, NB], BF, tag="o",
                                            name=f"o_{vi}_{b}_{j}")
                            nc.vector.tensor_copy(out=osb[:], in_=pss[j][:])
                            nc.sync.dma_start(
                                out=out_d[jg * 512 + j * P:
                                          jg * 512 + (j + 1) * P,
                                          b * NB:(b + 1) * NB],
                                in_=osb[:])

    nc.compile()
    return nc


_NC_CACHE = None


def _get_nc():
    global _NC_CACHE
    if _NC_CACHE is None:
        _NC_CACHE = build_nc()
    return _NC_CACHE


def _prep_inputs(x, hw0, hw1, hw2):
    """Host prep: tile x, build per-core periodic permuted-table slices."""
    x = np.asarray(x, np.float32)
    hws = [np.asarray(hw0, np.float32), np.asarray(hw1, np.float32),
           np.asarray(hw2, np.float32)]
    xT = np.ascontiguousarray(x.T).astype(ml_dtypes.bfloat16)  # [1024, 2048]
    # [128, BT, 8, 512]: xm[p, b, kt, :] = x[kt*128+p, b*512:(b+1)*512]
    xm = np.ascontiguousarray(
        xT.reshape(8, 128, BT, NB).transpose(1, 2, 0, 3).reshape(P, -1))

    # hb2 slice (device-side ladder source for W2)
    pl = PLANS[2]
    N2, b2 = pl["N"], pl["b"]
    t0 = pl["u0"] - pl["shift"]
    span = pl["m_ext"] + (N_CORES - 1) * JW[2]
    t = t0 + np.arange(span, dtype=np.int64)
    shared2 = hws[2][(b2 * t) % N2].astype(ml_dtypes.bfloat16)

    # host-materialized W0/W1 shards, tile-major wide layout
    def mat_w(l, jcols):
        i = np.arange(LENS[l], dtype=np.int64)[:, None]
        j = jcols[None, :]
        w = hws[l][(i * HASH_A[l] + j * HASH_B[l] + HASH_C[l]) % SIZES[l]]
        return w.astype(ml_dtypes.bfloat16)

    in_maps = []
    for c in range(N_CORES):
        w0 = mat_w(0, c * JW[0] + np.arange(JW[0], dtype=np.int64))
        w0m = np.ascontiguousarray(
            w0.reshape(8, 128, 512).transpose(1, 0, 2).reshape(128, 4096))
        # L1 is contraction-parallel: rows = this core's h1 columns
        # (natural), columns = ALL 4096 h2 columns in PERM2 order so the
        # ReduceScatter shards land in L2's ladder tile-major order
        i1 = (c * JW[1] + np.arange(JW[1], dtype=np.int64))[:, None]
        w1 = hws[1][(i1 * HASH_A[1] + PERM2[None, :] * HASH_B[1]
                     + HASH_C[1]) % SIZES[1]].astype(ml_dtypes.bfloat16)
        w1m = np.ascontiguousarray(w1.reshape(4, 128, 4096))
        in_maps.append({
            "xm": xm,
            "w0m": w0m,
            "w1m": w1m,
            "hb2": shared2[c * JW[2]: c * JW[2] + pl["m_ext"]],
        })
    return in_maps


def kernel(x, hw0, hw1, hw2, trace=False):
    nc = _get_nc()
    in_maps = _prep_inputs(x, hw0, hw1, hw2)
    res = run_bass_kernel_spmd(nc, in_maps, list(range(N_CORES)), trace=trace)
    outs = [np.asarray(res.results[c]["outT"][:JW[2], :])
            for c in range(N_CORES)]
    full = np.concatenate(outs, axis=0)         # [32000, 2048] bf16
    out = np.ascontiguousarray(full.T).astype(np.float32)
    kernel.last_results = res
    return out


# revision 19
# speedup vs baseline: 1.0470x; 1.0470x over previous
"""Hashed-weight MLP (1024-4096-4096-32000, batch 2048) on 8 TRN2 NeuronCores.

Problem: h = relu(x @ W0); h = relu(h @ W1); out = h @ W2, where each
W_l[i, j] = hw_l[(a_l*i + b_l*j + c_l) % N_l] is a virtual (ROBE-Z hashed)
weight gathered from a small parameter vector.

Approach (column-parallel tensor parallelism on all three layers):
  * Via the host-permuted table hb_bb[t] = hw[(b*t) % N] the virtual weight
    becomes row-contiguous: W[i, col] = hb_bb[shift + q*kk + r*c1 + col] with
    i = k*c1 + kk (q = b^-1 a, r = signed residue of q*k mod N). W2 tiles are
    DMAd straight from the per-core table slice into SBUF, 2-3 strided DMAs
    per 128-row tile (head run / kk-outer full-block rect / tail run).
  * Each core owns a 1/8 column shard of every layer; shard offsets are
    absorbed into the host-side slice (SPMD-uniform device program).
  * L2 uses uniform 128-row k-tiles whose within-tile row order is the
    ladder's kk-outer permutation. The matching rhs permutation is absorbed
    into WHICH h2 columns each core computes in L1 (host-materialized W1
    simply picks the permuted column subset), so the AllGathered h2 lands in
    DRAM already tile-major and h2res loads are plain 3-level strided DMAs.
  * L1 is contraction-parallel: each core multiplies its OWN L0 output
    (kept in SBUF, h1 never touches DRAM) against its 512-row slice of W1
    over all 4096 (permuted) output columns, then ReduceScatter+AllGather per
    batch tile sums the partials into the full pre-relu h2 on every core
    (relu applied later in-place on the SBUF-resident h2). This removes
    the h1 AllGathers entirely.
  * L2 keeps the whole 4096 x 2048 h2 activation resident in SBUF and
    streams W2 tiles through a ring.
  * Engines: scalar/sync = weight ladders + partial stores + output
    stores; vector/gpsimd = ReLU + PSUM evacuation (bf16 cast); gpsimd
    also triggers collectives and loads x/h2res. GEMMs are bf16 with
    fp32 PSUM; partials are reduced in bf16.
"""
import sys
if "/opt/trn_rl_repo" not in sys.path:
    sys.path.insert(0, "/opt/trn_rl_repo")

import numpy as np
import ml_dtypes

import concourse.bass as bass
import concourse.bacc as bacc
import concourse.tile as tile
import concourse.mybir as mybir
from concourse.bass_utils import run_bass_kernel_spmd

N_CORES = 8
P = 128
NB = 512                      # batch tile (matmul moving dim)
BATCH = 2048
BT = BATCH // NB              # 4

LENS = [1024, 4096, 4096, 32000]
HASH_A = [9973, 10007, 10039]
HASH_B = [31013, 31019, 31039]
HASH_C = [557, 563, 569]
SIZES = [1048576, 1048576, 4194304]
JW = [512, 512, 4000]         # true per-core output shard width
WTOT = [512, 512, 4096]       # max ladder col offset (L2 incl. jg offsets)

BF = mybir.dt.bfloat16
F32 = mybir.dt.float32


def _plan_layer(l):
    N = SIZES[l]; a, b, ch = HASH_A[l], HASH_B[l], HASH_C[l]
    binv = pow(b, -1, N)
    q = (binv * a) % N
    u0 = (binv * ch) % N
    in_dim = LENS[l]
    best = None
    for k in range(1, min(in_dim, 600) + 1):
        r = (q * k) % N
        if r > N // 2:
            r -= N
        C1 = -(-in_dim // k)
        extra = q * (k - 1) + abs(r) * (C1 - 1)
        if best is None or extra < best[0]:
            best = (extra, k, C1, r)
    _, k, C1, r = best
    shift = max(0, -r * (C1 - 1))
    m_ext = shift + q * (k - 1) + max(r, 0) * (C1 - 1) + WTOT[l] + 64
    return dict(N=N, b=b, q=q, u0=u0, k=k, r=r, shift=shift,
                m_ext=m_ext, in_dim=in_dim)


PLANS = [_plan_layer(l) for l in range(3)]
RG = [list(range(N_CORES))]


def _l2_tiles():
    """Uniform 128-row tiles for L2; seg = (kk0, c1_0, kc, c1c, p0).
    Within a multi-block seg the DMA streams kk-outer, so partition
    p = p0 + kkrel*c1c + c1rel."""
    k = PLANS[2]["k"]
    tiles = []
    for t in range(LENS[2] // P):
        i0, h = P * t, P
        segs = []
        row = i0
        kk0 = row % k
        if kk0:
            cnt = min(k - kk0, h)
            segs.append((kk0, row // k, cnt, 1, 0))
            row += cnt
        nfull = (i0 + h - row) // k
        if nfull:
            segs.append((0, row // k, k, nfull, row - i0))
            row += nfull * k
        if row < i0 + h:
            segs.append((0, row // k, i0 + h - row, 1, row - i0))
        tiles.append((i0, h, segs))
    return tiles


LT2 = _l2_tiles()
NKT2 = len(LT2)               # 32
NJG2 = 8                      # L2 j-groups of width 512 (4 j-tiles of 128)

def _l2_perm():
    """Global h2-row permutation: PERM[pos] = natural contraction row stored
    at tile-major position pos (tile t occupies positions [128t, 128t+128))."""
    perm = []
    for (i0, h, segs) in LT2:
        order = {}
        for (kk0, c1_0, kc, c1c, p0) in segs:
            for kk in range(kc):
                for c1r in range(c1c):
                    order[p0 + kk * c1c + c1r] = (c1_0 + c1r) * PLANS[2]["k"] \
                        + kk0 + kk
        perm.extend(order[p] for p in range(h))
    return np.array(perm, dtype=np.int64)


PERM2 = _l2_perm()


def _ladder_dmas(nc, eng, hb_t, l, wtile_ap, segs, col0, w):
    """Emit ladder DMAs for one weight tile of layer l into SBUF tile."""
    pl = PLANS[l]
    q, r, shift = pl["q"], pl["r"], pl["shift"]
    for (kk0, c1_0, kc, c1c, p0) in segs:
        if c1c == 1:
            src = bass.AP(hb_t, shift + q * kk0 + r * c1_0 + col0,
                          [[q, kc], [1, w]])
        else:
            src = bass.AP(hb_t, shift + q * kk0 + r * c1_0 + col0,
                          [[q, kc], [r, c1c], [1, w]])
        eng.dma_start(out=wtile_ap[p0:p0 + kc * c1c, :], in_=src)


def build_nc():
    nc = bacc.Bacc("TRN2", target_bir_lowering=False, debug=False,
                   num_devices=N_CORES)

    # x host-tiled as [128, BT*8*512]: row p holds, for each batch tile b
    # and k-tile kt, the 512 batch entries of x[kt*128+p, b*512:...], so a
    # per-(b) load is one plain 2D strided DMA with 8KB rows.
    xm_d = nc.dram_tensor("xm", [P, BT * 8 * NB], BF,
                          kind="ExternalInput").ap()
    # W0/W1 are host-materialized tile-major (wide rows -> 8KB descriptors);
    # only the big W2 still streams via the on-device hash ladder.
    w0m_d = nc.dram_tensor("w0m", [128, 4096], BF, kind="ExternalInput").ap()
    w1m_d = nc.dram_tensor("w1m", [4, 128, 4096], BF,
                           kind="ExternalInput").ap()
    hb2 = nc.dram_tensor("hb2", [PLANS[2]["m_ext"]], BF,
                         kind="ExternalInput").ap()
    # per-b partial h2 (pre-relu, PERM2 column order), its ReduceScatter
    # shard, and the AllGathered full pre-relu h2
    pt = [nc.dram_tensor(f"pt{b}", [4096, NB], BF).ap() for b in range(BT)]
    h2p = [nc.dram_tensor(f"h2p{b}", [512, NB], BF).ap() for b in range(BT)]
    h2f = [nc.dram_tensor(f"h2f{b}", [4096, NB], BF, addr_space="Shared").ap()
           for b in range(BT)]
    out_d = nc.dram_tensor("outT", [4096, BATCH], BF, kind="ExternalOutput").ap()

    with tile.TileContext(nc) as tc, \
         tc.tile_pool(name="ps", bufs=8, space="PSUM") as psp, \
         tc.tile_pool(name="resid", bufs=1) as residp:

        # h2 stays fully SBUF-resident for L2 (128 KB/partition); one wide
        # tile per batch tile, loaded in 4 chunks from h2f (whose rows are
        # already in ladder-permuted tile-major order).
        h2res = [residp.tile([P, NKT2 * NB], BF, name=f"h2r_{b}")
                 for b in range(BT)]

        def load_h2res(b):
            # on gpsimd (SWDGE): third DGE lane, keeps both HWDGE rings
            # free; 4 chunks so collective DMAs can interleave
            # h2f carries pre-relu sums; relu in place after each chunk.
            # b0/b1 relu on vector (free until L2's first psum copy);
            # b2/b3 on gpsimd so their late-AG waits never block the
            # vector queue ahead of L2's psum evacuations.
            reng = nc.vector if b < 2 else nc.gpsimd
            with nc.allow_non_contiguous_dma(reason="perm act load"):
                for c in range(4):
                    sl = slice(c * 8 * NB, (c + 1) * 8 * NB)
                    nc.gpsimd.dma_start(
                        out=h2res[b][:, sl],
                        in_=bass.AP(h2f[b].tensor, c * 8 * P * NB,
                                    [[NB, P], [P * NB, 8], [1, NB]]))
                    reng.tensor_scalar_max(
                        h2res[b][:, sl], h2res[b][:, sl], 0.0)

        # head of the first L2 slab: own (outer) pool so its ladders run
        # during L0/L1 without write-after-read waits on dying pools
        NHEAD = 6
        with nc.allow_non_contiguous_dma(reason="hash ladder"), \
             tc.tile_pool(name="jg0h", bufs=1) as jg0hp:
            jg0head = [jg0hp.tile([P, 512], BF, name=f"w2h{kt}")
                       for kt in range(NHEAD)]

            # w1sb lives below the L0 pools; its ladders prefetch during L0
            with tc.tile_pool(name="l1w", bufs=1) as l1wp:
                w1sb = [l1wp.tile([128, 4096], BF, name=f"w1sb{t}")
                        for t in range(4)]

                # ---- Layer 0 + contraction-parallel Layer 1 ----
                with tc.tile_pool(name="l0", bufs=1) as l0p, \
                     tc.tile_pool(name="l0x", bufs=2) as l0xp, \
                     tc.tile_pool(name="l0h", bufs=8) as l0hp, \
                     tc.tile_pool(name="l1o", bufs=8) as l1op:
                    w0sb = l0p.tile([128, 4096], BF, name="w0sb")

                    def load_x(b, eng):
                        xt = l0xp.tile([P, 8 * NB], BF, tag="x",
                                       name=f"x_{b}")
                        eng.dma_start(
                            out=xt[:],
                            in_=xm_d[:, b * 8 * NB:(b + 1) * 8 * NB])
                        return xt

                    # critical startup path first: w0 on scalar, x0 on sync
                    nc.scalar.dma_start(out=w0sb[:], in_=w0m_d[:, :])
                    xq = [load_x(0, nc.sync), load_x(1, nc.scalar)]
                    for t in range(4):
                        eng = nc.sync if t % 2 == 0 else nc.scalar
                        eng.dma_start(out=w1sb[t][:], in_=w1m_d[t, :, :])
                    for kt in range(NHEAD):
                        eng = nc.scalar if kt % 2 == 0 else nc.sync
                        _ladder_dmas(nc, eng, hb2.tensor, 2,
                                     jg0head[kt][:], LT2[kt][2], 0, 512)

                    for b in range(BT):
                        # L0(b): 4 j-tiles = L1's local contraction k-tiles;
                        # they stay in SBUF as L1's rhs (h1 never hits DRAM)
                        hq = []
                        for j in range(4):
                            ps = psp.tile([P, NB], F32, tag="ps",
                                          name=f"ps0_{b}_{j}")
                            for kt in range(8):
                                nc.tensor.matmul(
                                    out=ps[:],
                                    lhsT=w0sb[:, kt * 512 + j * P:
                                              kt * 512 + (j + 1) * P],
                                    rhs=xq[b][:, kt * NB:(kt + 1) * NB],
                                    start=(kt == 0), stop=(kt == 7))
                            hsb = l0hp.tile([P, NB], BF, tag="h1",
                                            name=f"h1_{b}_{j}")
                            nc.vector.tensor_scalar_max(hsb[:], ps[:], 0.0)
                            hq.append(hsb)
                        if b + 2 < BT:
                            xq.append(load_x(b + 2, nc.gpsimd))
                        # L1 partial over the local 512-row contraction
                        # slice, all 4096 PERM2-ordered output columns
                        for jt in range(32):
                            ps = psp.tile([P, NB], F32, tag="ps",
                                          name=f"ps1_{b}_{jt}")
                            for kt in range(4):
                                nc.tensor.matmul(
                                    out=ps[:],
                                    lhsT=w1sb[kt][:, jt * P:(jt + 1) * P],
                                    rhs=hq[kt][:],
                                    start=(kt == 0), stop=(kt == 3))
                            ob = l1op.tile([P, NB], BF, tag="l1o",
                                           name=f"p_{b}_{jt}")
                            nc.vector.tensor_copy(out=ob[:], in_=ps[:])
                            seng = nc.sync if jt % 2 == 0 else nc.scalar
                            seng.dma_start(
                                out=pt[b][jt * P:(jt + 1) * P, :], in_=ob[:])
                        # strict RS/AG alternation on the collective stream
                        # (AllReduce measured 71us/op vs RS~26 + AG~28):
                        # b0:[RS0] b1:[AG0,RS1] b2:[AG1,RS2] b3:[AG2,RS3]
                        if b >= 1:
                            nc.gpsimd.collective_compute(
                                "AllGather", mybir.AluOpType.bypass,
                                replica_groups=RG,
                                ins=[h2p[b - 1].opt()],
                                outs=[h2f[b - 1].opt()])
                        nc.gpsimd.collective_compute(
                            "ReduceScatter", mybir.AluOpType.add,
                            replica_groups=RG,
                            ins=[pt[b].opt()], outs=[h2p[b].opt()])
                        if b >= 2:
                            load_h2res(b - 2)

                    nc.gpsimd.collective_compute(
                        "AllGather", mybir.AluOpType.bypass,
                        replica_groups=RG,
                        ins=[h2p[BT - 1].opt()], outs=[h2f[BT - 1].opt()])
                    load_h2res(2)
                    load_h2res(3)

            # ---------------- Layer 2 ----------------
            # snake schedule: (jg0..jg7) x (b0,b1), then (jg7..jg0) x
            # (b2,b3). Only b0/b1's AllGather gates sit on the critical
            # path; b2/b3 arrive hundreds of us before first use. Each W2
            # slab is ladder-loaded twice (DMA has ample headroom), with
            # jg7's slab reused across the turn and jg0's head tiles
            # reused on both visits.
            runs = [(jg, (0, 1)) for jg in range(NJG2)] + \
                   [(jg, (2, 3)) for jg in reversed(range(NJG2))]
            with tc.tile_pool(name="w2", bufs=48) as w2p, \
                 tc.tile_pool(name="l2o", bufs=4) as l2op:
                slab = None
                last_jg = None
                for vi, (jg, bs) in enumerate(runs):
                    if jg != last_jg:
                        slab = []
                        for kt in range(NKT2):
                            if jg == 0 and kt < NHEAD:
                                slab.append(jg0head[kt])
                                continue
                            wt = w2p.tile([P, 512], BF, tag="w2t",
                                          name=f"w2_{vi}_{kt}")
                            weng = nc.scalar if kt % 2 == 0 else nc.sync
                            _ladder_dmas(nc, weng, hb2.tensor, 2, wt[:],
                                         LT2[kt][2], jg * 512, 512)
                            slab.append(wt)
                        last_jg = jg
                    for b in bs:
                        pss = [psp.tile([P, NB], F32, tag="ps",
                                        name=f"ps2_{vi}_{b}_{j}")
                               for j in range(4)]
                        for kt in range(NKT2):
                            for j in range(4):
                                nc.tensor.matmul(
                                    out=pss[j][:],
                                    lhsT=slab[kt][:, j * P:(j + 1) * P],
                                    rhs=h2res[b][:, kt * NB:(kt + 1) * NB],
                                    start=(kt == 0), stop=(kt == NKT2 - 1))
                        for j in range(4):
                            osb = l2op.tile([P, NB], BF, tag="o",
                                            name=f"o_{vi}_{b}_{j}")
                            nc.vector.tensor_copy(out=osb[:], in_=pss[j][:])
                            nc.sync.dma_start(
                                out=out_d[jg * 512 + j * P:
                                          jg * 512 + (j + 1) * P,
                                          b * NB:(b + 1) * NB],
                                in_=osb[:])

    nc.compile()
    return nc


_NC_CACHE = None


def _get_nc():
    global _NC_CACHE
    if _NC_CACHE is None:
        _NC_CACHE = build_nc()
    return _NC_CACHE


def _prep_inputs(x, hw0, hw1, hw2):
    """Host prep: tile x, build per-core periodic permuted-table slices."""
    x = np.asarray(x, np.float32)
    hws = [np.asarray(hw0, np.float32), np.asarray(hw1, np.float32),
           np.asarray(hw2, np.float32)]
    xT = np.ascontiguousarray(x.T).astype(ml_dtypes.bfloat16)  # [1024, 2048]
    # [128, BT, 8, 512]: xm[p, b, kt, :] = x[kt*128+p, b*512:(b+1)*512]
    xm = np.ascontiguousarray(
        xT.reshape(8, 128, BT, NB).transpose(1, 2, 0, 3).reshape(P, -1))

    # hb2 slice (device-side ladder source for W2)
    pl = PLANS[2]
    N2, b2 = pl["N"], pl["b"]
    t0 = pl["u0"] - pl["shift"]
    span = pl["m_ext"] + (N_CORES - 1) * JW[2]
    t = t0 + np.arange(span, dtype=np.int64)
    shared2 = hws[2][(b2 * t) % N2].astype(ml_dtypes.bfloat16)

    # host-materialized W0/W1 shards, tile-major wide layout
    def mat_w(l, jcols):
        i = np.arange(LENS[l], dtype=np.int64)[:, None]
        j = jcols[None, :]
        w = hws[l][(i * HASH_A[l] + j * HASH_B[l] + HASH_C[l]) % SIZES[l]]
        return w.astype(ml_dtypes.bfloat16)

    in_maps = []
    for c in range(N_CORES):
        w0 = mat_w(0, c * JW[0] + np.arange(JW[0], dtype=np.int64))
        w0m = np.ascontiguousarray(
            w0.reshape(8, 128, 512).transpose(1, 0, 2).reshape(128, 4096))
        # L1 is contraction-parallel: rows = this core's h1 columns
        # (natural), columns = ALL 4096 h2 columns in PERM2 order so the
        # ReduceScatter shards land in L2's ladder tile-major order
        i1 = (c * JW[1] + np.arange(JW[1], dtype=np.int64))[:, None]
        w1 = hws[1][(i1 * HASH_A[1] + PERM2[None, :] * HASH_B[1]
                     + HASH_C[1]) % SIZES[1]].astype(ml_dtypes.bfloat16)
        w1m = np.ascontiguousarray(w1.reshape(4, 128, 4096))
        in_maps.append({
            "xm": xm,
            "w0m": w0m,
            "w1m": w1m,
            "hb2": shared2[c * JW[2]: c * JW[2] + pl["m_ext"]],
        })
    return in_maps


def kernel(x, hw0, hw1, hw2, trace=False):
    nc = _get_nc()
    in_maps = _prep_inputs(x, hw0, hw1, hw2)
    res = run_bass_kernel_spmd(nc, in_maps, list(range(N_CORES)), trace=trace)
    outs = [np.asarray(res.results[c]["outT"][:JW[2], :])
            for c in range(N_CORES)]
    full = np.concatenate(outs, axis=0)         # [32000, 2048] bf16
    out = np.ascontiguousarray(full.T).astype(np.float32)
    kernel.last_results = res
    return out


# revision 20
# speedup vs baseline: 1.0964x; 1.0472x over previous
"""Hashed-weight MLP (1024-4096-4096-32000, batch 2048) on 8 TRN2 NeuronCores.

Problem: h = relu(x @ W0); h = relu(h @ W1); out = h @ W2, where each
W_l[i, j] = hw_l[(a_l*i + b_l*j + c_l) % N_l] is a virtual (ROBE-Z hashed)
weight gathered from a small parameter vector.

Approach (column-parallel tensor parallelism on all three layers):
  * Via the host-permuted table hb_bb[t] = hw[(b*t) % N] the virtual weight
    becomes row-contiguous: W[i, col] = hb_bb[shift + q*kk + r*c1 + col] with
    i = k*c1 + kk (q = b^-1 a, r = signed residue of q*k mod N). W2 tiles are
    DMAd straight from the per-core table slice into SBUF, 2-3 strided DMAs
    per 128-row tile (head run / kk-outer full-block rect / tail run).
  * Each core owns a 1/8 column shard of every layer; shard offsets are
    absorbed into the host-side slice (SPMD-uniform device program).
  * L2 uses uniform 128-row k-tiles whose within-tile row order is the
    ladder's kk-outer permutation. The matching rhs permutation is absorbed
    into WHICH h2 columns each core computes in L1 (host-materialized W1
    simply picks the permuted column subset), so the AllGathered h2 lands in
    DRAM already tile-major and h2res loads are plain 3-level strided DMAs.
  * AllGathers only (ReduceScatter/AllReduce measured 45-77us vs AG 27-40us
    for the same payload on this stack). Each AG op costs ~16us fixed +
    ~2.9us/MB, so b2+b3's h1 AGs are paired into one double-width op while
    h2 AGs stay solo per batch tile (their per-b arrival gates L2).
  * L2 runs a snake schedule ((jg0..jg7) x (b0,b1), then (jg7..jg0) x
    (b2,b3)): only b0/b1's AllGather gates sit on the critical path; b2/b3
    arrive hundreds of us before first use. Each W2 slab is ladder-loaded
    twice (DMA has ample headroom), with jg7's slab reused across the turn
    and jg0's head tiles reused on both visits.
  * L2 keeps the whole 4096 x 2048 h2 activation resident in SBUF.
  * Engines: scalar/sync = weight ladders + L1 rhs chunks + output stores;
    vector = ReLU + PSUM evacuation (bf16 cast); gpsimd = x/h2res loads,
    activation stores, AllGather triggers. GEMMs are bf16 with fp32 PSUM.
"""
import sys
if "/opt/trn_rl_repo" not in sys.path:
    sys.path.insert(0, "/opt/trn_rl_repo")

import numpy as np
import ml_dtypes

import concourse.bass as bass
import concourse.bacc as bacc
import concourse.tile as tile
import concourse.mybir as mybir
from concourse.bass_utils import run_bass_kernel_spmd

N_CORES = 8
P = 128
NB = 512                      # batch tile (matmul moving dim)
BATCH = 2048
BT = BATCH // NB              # 4

LENS = [1024, 4096, 4096, 32000]
HASH_A = [9973, 10007, 10039]
HASH_B = [31013, 31019, 31039]
HASH_C = [557, 563, 569]
SIZES = [1048576, 1048576, 4194304]
JW = [512, 512, 4000]         # true per-core output shard width
WTOT = [512, 512, 4096]       # max ladder col offset (L2 incl. jg offsets)

BF = mybir.dt.bfloat16
F32 = mybir.dt.float32


def _plan_layer(l):
    N = SIZES[l]; a, b, ch = HASH_A[l], HASH_B[l], HASH_C[l]
    binv = pow(b, -1, N)
    q = (binv * a) % N
    u0 = (binv * ch) % N
    in_dim = LENS[l]
    best = None
    for k in range(1, min(in_dim, 600) + 1):
        r = (q * k) % N
        if r > N // 2:
            r -= N
        C1 = -(-in_dim // k)
        extra = q * (k - 1) + abs(r) * (C1 - 1)
        if best is None or extra < best[0]:
            best = (extra, k, C1, r)
    _, k, C1, r = best
    shift = max(0, -r * (C1 - 1))
    m_ext = shift + q * (k - 1) + max(r, 0) * (C1 - 1) + WTOT[l] + 64
    return dict(N=N, b=b, q=q, u0=u0, k=k, r=r, shift=shift,
                m_ext=m_ext, in_dim=in_dim)


PLANS = [_plan_layer(l) for l in range(3)]
RG = [list(range(N_CORES))]


def _l2_tiles():
    """Uniform 128-row tiles for L2; seg = (kk0, c1_0, kc, c1c, p0).
    Within a multi-block seg the DMA streams kk-outer, so partition
    p = p0 + kkrel*c1c + c1rel."""
    k = PLANS[2]["k"]
    tiles = []
    for t in range(LENS[2] // P):
        i0, h = P * t, P
        segs = []
        row = i0
        kk0 = row % k
        if kk0:
            cnt = min(k - kk0, h)
            segs.append((kk0, row // k, cnt, 1, 0))
            row += cnt
        nfull = (i0 + h - row) // k
        if nfull:
            segs.append((0, row // k, k, nfull, row - i0))
            row += nfull * k
        if row < i0 + h:
            segs.append((0, row // k, i0 + h - row, 1, row - i0))
        tiles.append((i0, h, segs))
    return tiles


LT2 = _l2_tiles()
NKT2 = len(LT2)               # 32
NJG2 = 8                      # L2 j-groups of width 512 (4 j-tiles of 128)


def _l2_perm():
    """Global h2-row permutation: PERM[pos] = natural contraction row stored
    at tile-major position pos (tile t occupies positions [128t, 128t+128))."""
    perm = []
    for (i0, h, segs) in LT2:
        order = {}
        for (kk0, c1_0, kc, c1c, p0) in segs:
            for kk in range(kc):
                for c1r in range(c1c):
                    order[p0 + kk * c1c + c1r] = (c1_0 + c1r) * PLANS[2]["k"] \
                        + kk0 + kk
        perm.extend(order[p] for p in range(h))
    return np.array(perm, dtype=np.int64)


PERM2 = _l2_perm()


def _ladder_dmas(nc, eng, hb_t, l, wtile_ap, segs, col0, w):
    """Emit ladder DMAs for one weight tile of layer l into SBUF tile."""
    pl = PLANS[l]
    q, r, shift = pl["q"], pl["r"], pl["shift"]
    for (kk0, c1_0, kc, c1c, p0) in segs:
        if c1c == 1:
            src = bass.AP(hb_t, shift + q * kk0 + r * c1_0 + col0,
                          [[q, kc], [1, w]])
        else:
            src = bass.AP(hb_t, shift + q * kk0 + r * c1_0 + col0,
                          [[q, kc], [r, c1c], [1, w]])
        eng.dma_start(out=wtile_ap[p0:p0 + kc * c1c, :], in_=src)


def build_nc():
    nc = bacc.Bacc("TRN2", target_bir_lowering=False, debug=False,
                   num_devices=N_CORES)

    # x host-tiled as [128, BT*8*512]: row p holds, for each batch tile b
    # and k-tile kt, the 512 batch entries of x[kt*128+p, b*512:...], so a
    # per-(b) load is one plain 2D strided DMA with 8KB rows.
    xm_d = nc.dram_tensor("xm", [P, BT * 8 * NB], BF,
                          kind="ExternalInput").ap()
    # W0/W1 are host-materialized tile-major (wide rows -> 8KB descriptors);
    # only the big W2 still streams via the on-device hash ladder.
    w0m_d = nc.dram_tensor("w0m", [128, 4096], BF, kind="ExternalInput").ap()
    w1m_d = nc.dram_tensor("w1m", [4, 128, 4096], BF,
                           kind="ExternalInput").ap()
    hb2 = nc.dram_tensor("hb2", [PLANS[2]["m_ext"]], BF,
                         kind="ExternalInput").ap()
    # b2+b3 share double-width tensors so their AllGather is one op
    # (each AG op has ~16us fixed cost; fewer, larger ops win)
    h1c = [nc.dram_tensor("h1c0", [512, NB], BF).ap(),
           nc.dram_tensor("h1c1", [512, NB], BF).ap(),
           nc.dram_tensor("h1c23", [512, 2 * NB], BF).ap()]
    h1f = [nc.dram_tensor("h1f0", [4096, NB], BF, addr_space="Shared").ap(),
           nc.dram_tensor("h1f1", [4096, NB], BF, addr_space="Shared").ap(),
           nc.dram_tensor("h1f23", [4096, 2 * NB], BF,
                          addr_space="Shared").ap()]
    h2c = [nc.dram_tensor(f"h2c{b}", [512, NB], BF).ap() for b in range(BT)]
    h2f = [nc.dram_tensor(f"h2f{b}", [4096, NB], BF, addr_space="Shared").ap()
           for b in range(BT)]

    def grp(b):
        # (tensor index, column offset, row width) for batch tile b
        return (b, 0, NB) if b < 2 else (2, (b - 2) * NB, 2 * NB)
    out_d = nc.dram_tensor("outT", [4096, BATCH], BF, kind="ExternalOutput").ap()

    with tile.TileContext(nc) as tc, \
         tc.tile_pool(name="ps", bufs=8, space="PSUM") as psp, \
         tc.tile_pool(name="resid", bufs=1) as residp:

        # h2 stays fully SBUF-resident for L2 (128 KB/partition); one wide
        # tile per batch tile, loaded in 4 chunks from h2f (whose rows are
        # already in ladder-permuted tile-major order).
        h2res = [residp.tile([P, NKT2 * NB], BF, name=f"h2r_{b}")
                 for b in range(BT)]

        def load_h2res(b):
            # on gpsimd (SWDGE): third DGE lane, keeps both HWDGE rings
            # free; 4 chunks so collective DMAs can interleave
            with nc.allow_non_contiguous_dma(reason="perm act load"):
                for c in range(4):
                    nc.gpsimd.dma_start(
                        out=h2res[b][:, c * 8 * NB:(c + 1) * 8 * NB],
                        in_=bass.AP(h2f[b].tensor, c * 8 * P * NB,
                                    [[NB, P], [P * NB, 8], [1, NB]]))

        # head of the first L2 slab: own (outer) pool so its ladders run
        # during L0/L1 without write-after-read waits on dying pools
        NHEAD = 18
        with nc.allow_non_contiguous_dma(reason="hash ladder"), \
             tc.tile_pool(name="jg0h", bufs=1) as jg0hp:
            jg0head = [jg0hp.tile([P, 512], BF, name=f"w2h{kt}")
                       for kt in range(NHEAD)]

            # w1sb lives below the L0 pools; its ladders prefetch during L0
            with tc.tile_pool(name="l1w", bufs=1) as l1wp:
                w1sb = [l1wp.tile([128, 4096], BF, name=f"w1sb{t}")
                        for t in range(4)]

                # ---------------- Layer 0 ----------------
                with tc.tile_pool(name="l0", bufs=1) as l0p, \
                     tc.tile_pool(name="l0x", bufs=2) as l0xp, \
                     tc.tile_pool(name="l0h", bufs=4) as l0hp:
                    w0sb = l0p.tile([128, 4096], BF, name="w0sb")

                    def load_x(b, eng):
                        xt = l0xp.tile([P, 8 * NB], BF, tag="x",
                                       name=f"x_{b}")
                        eng.dma_start(
                            out=xt[:],
                            in_=xm_d[:, b * 8 * NB:(b + 1) * 8 * NB])
                        return xt

                    # critical startup path first: w0 on scalar, x0 on sync
                    nc.scalar.dma_start(out=w0sb[:], in_=w0m_d[:, :])
                    xq = [load_x(0, nc.sync), load_x(1, nc.scalar)]
                    for t in range(4):
                        eng = nc.sync if t % 2 == 0 else nc.scalar
                        eng.dma_start(out=w1sb[t][:], in_=w1m_d[t, :, :])
                    for kt in range(NHEAD):
                        eng = nc.scalar if kt % 2 == 0 else nc.sync
                        _ladder_dmas(nc, eng, hb2.tensor, 2,
                                     jg0head[kt][:], LT2[kt][2], 0, 512)

                    for b in range(BT):
                        for j in range(4):
                            ps = psp.tile([P, NB], F32, tag="ps",
                                          name=f"ps0_{b}_{j}")
                            for kt in range(8):
                                nc.tensor.matmul(
                                    out=ps[:],
                                    lhsT=w0sb[:, kt * 512 + j * P:
                                              kt * 512 + (j + 1) * P],
                                    rhs=xq[b][:, kt * NB:(kt + 1) * NB],
                                    start=(kt == 0), stop=(kt == 7))
                            hsb = l0hp.tile([P, NB], BF, tag="h1",
                                            name=f"h1_{b}_{j}")
                            nc.vector.tensor_scalar_max(hsb[:], ps[:], 0.0)
                            gi, cofs, _ = grp(b)
                            nc.gpsimd.dma_start(
                                out=h1c[gi][j * P:(j + 1) * P,
                                            cofs:cofs + NB],
                                in_=hsb[:])
                        if b != 2:
                            gi = grp(b)[0]
                            nc.gpsimd.collective_compute(
                                "AllGather", mybir.AluOpType.bypass,
                                replica_groups=RG,
                                ins=[h1c[gi].opt()], outs=[h1f[gi].opt()])
                        if b + 2 < BT:
                            xq.append(load_x(b + 2, nc.gpsimd))

                # ---------------- Layer 1 ----------------
                with tc.tile_pool(name="l1r", bufs=2) as l1rp, \
                     tc.tile_pool(name="l1h", bufs=8) as l1hp:
                    for b in range(BT):
                        pss = [psp.tile([P, NB], F32, tag="ps",
                                        name=f"ps1_{b}_{j}")
                               for j in range(4)]
                        for cc in range(4):
                            rhs = l1rp.tile([P, 8 * NB], BF, tag="l1rhs",
                                            name=f"l1r_{b}_{cc}")
                            # two parallel half-loads (sync+scalar) halve
                            # the post-AllGather rhs latency
                            with nc.allow_non_contiguous_dma(
                                    reason="h1 chunk gather"):
                                gi, cofs, rw = grp(b)
                                for hh, heng in ((0, nc.sync),
                                                 (1, nc.scalar)):
                                    heng.dma_start(
                                        out=rhs[:, hh * 4 * NB:
                                                (hh + 1) * 4 * NB],
                                        in_=bass.AP(h1f[gi].tensor,
                                                    (cc * 8 + hh * 4)
                                                    * P * rw + cofs,
                                                    [[rw, P], [P * rw, 4],
                                                     [1, NB]]))
                            for kts in range(8):
                                kt = cc * 8 + kts
                                co = (kt % 8) * 512
                                for j in range(4):
                                    nc.tensor.matmul(
                                        out=pss[j][:],
                                        lhsT=w1sb[kt // 8][:, co + j * P:
                                                           co + (j + 1) * P],
                                        rhs=rhs[:, kts * NB:(kts + 1) * NB],
                                        start=(kt == 0), stop=(kt == 31))
                        for j in range(4):
                            hsb = l1hp.tile([P, NB], BF, tag="h2",
                                            name=f"h2_{b}_{j}")
                            nc.vector.tensor_scalar_max(hsb[:], pss[j][:], 0.0)
                            nc.gpsimd.dma_start(
                                out=h2c[b][j * P:(j + 1) * P, :], in_=hsb[:])
                        nc.gpsimd.collective_compute(
                            "AllGather", mybir.AluOpType.bypass,
                            replica_groups=RG,
                            ins=[h2c[b].opt()], outs=[h2f[b].opt()])
                        # residency loads for AGs that finished earlier;
                        # emitted after this b's stores+AG so they don't
                        # delay the store->AG chain on the gpsimd queue
                        if b >= 2:
                            load_h2res(b - 2)

                    load_h2res(2)
                    load_h2res(3)

            # ---------------- Layer 2 ----------------
            # snake schedule: (jg0..jg7) x (b0,b1), then (jg7..jg0) x
            # (b2,b3). Only b0/b1's AllGather gates sit on the critical
            # path; b2/b3 arrive hundreds of us before first use.
            runs = [(jg, (0, 1)) for jg in range(NJG2)] + \
                   [(jg, (2, 3)) for jg in reversed(range(NJG2))]
            with tc.tile_pool(name="w2", bufs=48) as w2p, \
                 tc.tile_pool(name="l2o", bufs=4) as l2op:
                slab = None
                last_jg = None
                for vi, (jg, bs) in enumerate(runs):
                    if jg != last_jg:
                        slab = []
                        for kt in range(NKT2):
                            if jg == 0 and kt < NHEAD:
                                slab.append(jg0head[kt])
                                continue
                            wt = w2p.tile([P, 512], BF, tag="w2t",
                                          name=f"w2_{vi}_{kt}")
                            weng = nc.scalar if kt % 2 == 0 else nc.sync
                            _ladder_dmas(nc, weng, hb2.tensor, 2, wt[:],
                                         LT2[kt][2], jg * 512, 512)
                            slab.append(wt)
                        last_jg = jg
                    for b in bs:
                        pss = [psp.tile([P, NB], F32, tag="ps",
                                        name=f"ps2_{vi}_{b}_{j}")
                               for j in range(4)]
                        for kt in range(NKT2):
                            for j in range(4):
                                nc.tensor.matmul(
                                    out=pss[j][:],
                                    lhsT=slab[kt][:, j * P:(j + 1) * P],
                                    rhs=h2res[b][:, kt * NB:(kt + 1) * NB],
                                    start=(kt == 0), stop=(kt == NKT2 - 1))
                        for j in range(4):
                            osb = l2op.tile([P, NB], BF, tag="o",
                                            name=f"o_{vi}_{b}_{j}")
                            nc.vector.tensor_copy(out=osb[:], in_=pss[j][:])
                            nc.sync.dma_start(
                                out=out_d[jg * 512 + j * P:
                                          jg * 512 + (j + 1) * P,
                                          b * NB:(b + 1) * NB],
                                in_=osb[:])

    nc.compile()
    return nc


_NC_CACHE = None


def _get_nc():
    global _NC_CACHE
    if _NC_CACHE is None:
        _NC_CACHE = build_nc()
    return _NC_CACHE


def _prep_inputs(x, hw0, hw1, hw2):
    """Host prep: tile x, build per-core periodic permuted-table slices."""
    x = np.asarray(x, np.float32)
    hws = [np.asarray(hw0, np.float32), np.asarray(hw1, np.float32),
           np.asarray(hw2, np.float32)]
    xT = np.ascontiguousarray(x.T).astype(ml_dtypes.bfloat16)  # [1024, 2048]
    # [128, BT, 8, 512]: xm[p, b, kt, :] = x[kt*128+p, b*512:(b+1)*512]
    xm = np.ascontiguousarray(
        xT.reshape(8, 128, BT, NB).transpose(1, 2, 0, 3).reshape(P, -1))

    # hb2 slice (device-side ladder source for W2)
    pl = PLANS[2]
    N2, b2 = pl["N"], pl["b"]
    t0 = pl["u0"] - pl["shift"]
    span = pl["m_ext"] + (N_CORES - 1) * JW[2]
    t = t0 + np.arange(span, dtype=np.int64)
    shared2 = hws[2][(b2 * t) % N2].astype(ml_dtypes.bfloat16)

    # host-materialized W0/W1 shards, tile-major wide layout
    def mat_w(l, jcols):
        i = np.arange(LENS[l], dtype=np.int64)[:, None]
        j = jcols[None, :]
        w = hws[l][(i * HASH_A[l] + j * HASH_B[l] + HASH_C[l]) % SIZES[l]]
        return w.astype(ml_dtypes.bfloat16)

    in_maps = []
    for c in range(N_CORES):
        w0 = mat_w(0, c * JW[0] + np.arange(JW[0], dtype=np.int64))
        w0m = np.ascontiguousarray(
            w0.reshape(8, 128, 512).transpose(1, 0, 2).reshape(128, 4096))
        # L1 columns = the h2 rows this core's AG shard must hold so that
        # the concatenated h2f is in L2's ladder tile-major order
        w1 = mat_w(1, PERM2[c * JW[1]: (c + 1) * JW[1]])
        w1m = np.ascontiguousarray(
            w1.reshape(4, 8, 128, 512).transpose(0, 2, 1, 3)
              .reshape(4, 128, 4096))
        in_maps.append({
            "xm": xm,
            "w0m": w0m,
            "w1m": w1m,
            "hb2": shared2[c * JW[2]: c * JW[2] + pl["m_ext"]],
        })
    return in_maps


def kernel(x, hw0, hw1, hw2, trace=False):
    nc = _get_nc()
    in_maps = _prep_inputs(x, hw0, hw1, hw2)
    res = run_bass_kernel_spmd(nc, in_maps, list(range(N_CORES)), trace=trace)
    outs = [np.asarray(res.results[c]["outT"][:JW[2], :])
            for c in range(N_CORES)]
    full = np.concatenate(outs, axis=0)         # [32000, 2048] bf16
    out = np.ascontiguousarray(full.T).astype(np.float32)
    kernel.last_results = res
    return out
